# revision 13
# baseline (speedup 1.0000x reference)
"""Multi-head causal attention on 8 Trainium2 NeuronCores.

Sharding: core = (batch b in {0,1}) x (head-group g in {0..3}); each core
computes 4 of the 16 heads for one batch element and returns a partial
(d_model, n) bf16 output (its heads' contribution to the final projection,
transposed). Host sums the 4 partials per batch (w_o row-parallel reduce),
transposes, and stacks.

Per-core pipeline (all engines load-balanced):
  1. Q/K projections into transposed head-pair tiles [he(64)|ho(64), n]
     (PSUM->SBUF copies on ScalarE, idle in this phase). V projection in
     natural layout into per-(head, j-block) augmented tiles [128, 65]
     (ones column = softmax denominator accumulator), copies on VectorE.
  2. Attention per head-pair, i-chunk outer / j-block inner. Scores for the
     two heads are independent K=64 matmuls into the two halves of one
     [128, 1024] PSUM tile (row-tiled: even head rows 0-63 at tile position
     (0,0), odd head rows 64-127 at (64,0) -- they run concurrently on HW).
     exp() on the whole [128, 1024] tile, split between ScalarE (true exp)
     and VectorE (Schraudolph bf16 bit-trick: bits = round(lam*s + C)
     computed f32 -> int16, bitcast as bf16). Causal masking by tri-mask
     multiply on the diagonal block. AV accumulates [65, 512] per (head,
     i-chunk) over j; normalization by broadcast reciprocal of row 64.
  3. Output projection (w_o stationary), bf16 output staged per 128-row
     block and DMA'd on the sync ring.
"""

import math
import os

import numpy as np

H = 16
D_MODEL = 1024
D_K = 64
N = 2048
B = 2
N_CORES = 8
N_GROUPS = 4          # head groups (tensor parallel)
HPC = H // N_GROUPS   # heads per core = 4
GD = HPC * D_K        # group output dim = 256
EXP_SCALE = 1.0 / math.sqrt(D_K)
EXP_BIAS = -5.0

# Schraudolph bf16 fast-exp (VectorE path):
#   bf16_bits(exp(s/8 - 5)) ~= round(LAM * s + CEXP), zero-mean sigma.
_LOG2E = 1.0 / math.log(2.0)
_SIGMA = 0.0579
LAM = 128.0 * _LOG2E / 8.0
CEXP = 128.0 * (127.0 - _SIGMA) + 128.0 * _LOG2E * EXP_BIAS

_DT = os.environ.get("BASS_MHA_DT", "bf16")


def _build(dt_name: str, n_iters: int = 1):
    """Emit and compile the single-core SPMD program. Returns compiled nc."""
    import concourse.bacc as bacc
    import concourse.mybir as mybir
    import concourse.tile as tile

    # fraction of exp tiles routed to VectorE (Schraudolph), as i%denom<num
    expsplit = os.environ.get("BASS_MHA_EXPSPLIT", "1/4")
    es_num, es_den = (int(x) for x in expsplit.split("/"))
    tri_eng = os.environ.get("BASS_MHA_TRIENG", "vector")

    dt = {"bf16": mybir.dt.bfloat16, "f32r": mybir.dt.float32r}[dt_name]
    f32 = mybir.dt.float32
    i16 = mybir.dt.int16

    nc = bacc.Bacc("TRN2", num_devices=N_CORES)

    xqT = nc.dram_tensor("xqT", [D_MODEL, N], dt, kind="ExternalInput").ap()
    xkT = nc.dram_tensor("xkT", [D_MODEL, N], dt, kind="ExternalInput").ap()
    xvT = nc.dram_tensor("xvT", [D_MODEL, N], dt, kind="ExternalInput").ap()
    wqT = nc.dram_tensor("wqT", [D_MODEL, GD], dt, kind="ExternalInput").ap()
    wkT = nc.dram_tensor("wkT", [D_MODEL, GD], dt, kind="ExternalInput").ap()
    wvT = nc.dram_tensor("wvT", [D_MODEL, GD], dt, kind="ExternalInput").ap()
    woT = nc.dram_tensor("woT", [GD, D_MODEL], dt, kind="ExternalInput").ap()
    tri = nc.dram_tensor("tri", [128, 128], dt, kind="ExternalInput").ap()
    outT = nc.dram_tensor("outT", [D_MODEL, N], dt, kind="ExternalOutput").ap()

    KC = D_MODEL // 128   # 8 contraction chunks
    NI = N // 512         # 4 i-chunks of 512
    NJ = N // 128         # 16 j-chunks of 128

    xq_t = xqT.rearrange("(kc p) i -> kc p i", p=128)
    xk_t = xkT.rearrange("(kc p) i -> kc p i", p=128)
    xv_t = xvT.rearrange("(kc p) i -> kc p i", p=128)
    wq_t = wqT.rearrange("(kc p) m -> kc p m", p=128)
    wk_t = wkT.rearrange("(kc p) m -> kc p m", p=128)
    wv_t = wvT.rearrange("(kc p) m -> kc p m", p=128)
    wo_t = woT.rearrange("(oc p) m -> oc p m", p=128)
    outT_t = outT.rearrange("(ms p) i -> ms p i", p=128)

    from contextlib import ExitStack

    with tile.TileContext(nc) as tc, ExitStack() as ctx:
        sb_w = ctx.enter_context(tc.tile_pool(name="weights", bufs=1))
        sb_p = ctx.enter_context(tc.tile_pool(name="persist", bufs=1))
        sb_e = ctx.enter_context(tc.tile_pool(name="expw", bufs=4))
        sb_o = ctx.enter_context(tc.tile_pool(name="outw", bufs=3))

        # ---- persistent tiles (alive across loop iterations) ----
        wq_s = [sb_w.tile([128, GD], dt, tag=f"wq{k}", name=f"wq{k}") for k in range(KC)]
        wk_s = [sb_w.tile([128, GD], dt, tag=f"wk{k}", name=f"wk{k}") for k in range(KC)]
        wv_s = [sb_w.tile([128, GD], dt, tag=f"wv{k}", name=f"wv{k}") for k in range(KC)]
        wo_s = [sb_w.tile([128, D_MODEL], dt, tag=f"wo{o}", name=f"wo{o}") for o in range(2)]
        tri_s = sb_w.tile([128, 128], dt, tag="tri")
        ebias = sb_w.tile([128, 1], f32, tag="ebias")
        # xv stays resident (V proj reads every chunk once per n-block round)
        xv_s = [sb_p.tile([128, N], dt, tag=f"xv{k}", name=f"xv{k}") for k in range(KC)]
        # xk/xq stream through small rotating pools (cross-iter DMA prefetch)
        sb_x = ctx.enter_context(tc.tile_pool(name="xin", bufs=3))
        # KhT / QhT pair tiles: [(h_even d64 | h_odd d64), n]
        kh = [sb_p.tile([128, N], dt, tag=f"kh{m}", name=f"kh{m}") for m in range(2)]
        qp = [sb_p.tile([128, N], dt, tag=f"qp{m}", name=f"qp{m}") for m in range(2)]
        # Vaug per (head, j-chunk): [128 j, 65], col 64 = 1.0
        va = [[sb_p.tile([128, 65], dt, tag=f"va{h}_{nt}", name=f"va{h}{nt}")
               for nt in range(NJ)] for h in range(HPC)]
        # normalized O^T per pair: [(h_even d64 | h_odd d64), n]
        ot = [sb_p.tile([128, N], dt, tag=f"ot{p}", name=f"ot{p}") for p in range(2)]

        # ---- loop-invariant init (outside For_i) ----
        nc.vector.memset(ebias[:], EXP_BIAS)
        for h in range(HPC):
            for nt in range(NJ):
                ones_ap = va[h][nt][:, 64:65]
                if dt == mybir.dt.float32r:
                    nc.vector.memset(ones_ap.bitcast(f32), 1.0)
                else:
                    nc.vector.memset(ones_ap, 1.0)

        def body():
            # ---- DMAs: weights on pool ring; x on sync ring, q,k,v order --
            for k in range(KC):
                nc.gpsimd.dma_start(wk_s[k][:], wk_t[k])
                nc.gpsimd.dma_start(wq_s[k][:], wq_t[k])
                nc.gpsimd.dma_start(wv_s[k][:], wv_t[k])
            nc.gpsimd.dma_start(tri_s[:], tri[:])
            nc.gpsimd.dma_start(wo_s[0][:], wo_t[0])
            nc.gpsimd.dma_start(wo_s[1][:], wo_t[1])
            xk_r = [sb_x.tile([128, N], dt, tag="xk", name=f"xk{k}")
                    for k in range(KC)]
            for k in range(KC):
                nc.sync.dma_start(xk_r[k][:], xk_t[k])
            xq_r = [sb_x.tile([128, N], dt, tag="xq", name=f"xq{k}")
                    for k in range(KC)]
            for k in range(KC):
                nc.sync.dma_start(xq_r[k][:], xq_t[k])
            for k in range(KC):
                nc.sync.dma_start(xv_s[k][:], xv_t[k])

            # ========== Phase 1a: K/Q projections (pair-tile layout) =======
            with tc.tile_pool(name="ps1", bufs=1, space="PSUM") as ps1:
                for xb, ws, dest in ((xk_r, wk_s, kh), (xq_r, wq_s, qp)):
                    pt = [[ps1.tile([128, 512], f32, tag=f"proj{m}{i}",
                                    name=f"pt{m}{i}")
                           for i in range(NI)] for m in range(2)]
                    for k in range(KC):
                        for m in range(2):
                            for i in range(NI):
                                nc.tensor.matmul(
                                    pt[m][i][:],
                                    ws[k][:, m * 128:(m + 1) * 128],
                                    xb[k][:, i * 512:(i + 1) * 512],
                                    start=(k == 0), stop=(k == KC - 1),
                                )
                    for m in range(2):
                        for i in range(NI):
                            nc.scalar.copy(
                                dest[m][:, i * 512:(i + 1) * 512], pt[m][i][:])

            # == Phase 1b + 2 share one PSUM region (V proj | attention) ====
            # banks: acc tags (pv/po shared) 2x2 + seo 2x2 = 8
            exp_idx = 0
            with tc.tile_pool(name="ps2", bufs=1, space="PSUM") as ps2:
                def acc_pair():
                    return [ps2.tile([128, 512], f32, tag=f"acc{u}",
                                     name=f"acc{u}", bufs=2) for u in range(2)]

                # ---- V projection: 2 n-chunks per round ------------------
                for blk in range(NJ // 2):
                    pv = acc_pair()
                    for k in range(KC):
                        for i in range(2):
                            nt = blk * 2 + i
                            nc.tensor.matmul(
                                pv[i][:, 0:GD],
                                xv_s[k][:, nt * 128:(nt + 1) * 128],
                                wv_s[k][:],
                                start=(k == 0), stop=(k == KC - 1),
                            )
                    for i in range(2):
                        nt = blk * 2 + i
                        for h in range(HPC):
                            nc.vector.tensor_copy(
                                va[h][nt][:, 0:64],
                                pv[i][:, 64 * h:64 * (h + 1)],
                            )

                # ---- attention: i-chunk outer, AV lags scores by 2 -------
                ets = {}
                po_cur = {}

                def av_emit(p, c, J):
                    off = 128 * (J - 4 * c) if J >= 4 * c else 0
                    et = ets.pop((p, c, J))
                    if J == 0:
                        po_cur["t"] = acc_pair()
                    po = po_cur["t"]
                    for e in range(2):
                        nc.tensor.matmul(
                            po[e][0:65, off:],
                            va[2 * p + e][J][:],
                            et[:, 512 * e + off:512 * (e + 1)],
                            start=(J == 0), stop=(J == 4 * c + 3),
                            skip_group_check=True,
                        )
                    if J == 4 * c + 3:  # chunk complete: normalize
                        for e in range(2):
                            rec = sb_o.tile([1, 512], f32, tag="rec")
                            nc.vector.reciprocal(rec[:], po[e][64:65, :])
                            rb = sb_o.tile([64, 512], f32, tag="rb")
                            nc.gpsimd.partition_broadcast(rb[:], rec[0:1, :])
                            nc.vector.tensor_mul(
                                ot[p][64 * e:64 * (e + 1),
                                      c * 512:(c + 1) * 512],
                                po[e][0:64, :], rb[:],
                            )

                pending = []
                for p in range(2):
                    for c in range(NI):
                        for J in range(4 * c + 4):
                            off = 128 * (J - 4 * c) if J >= 4 * c else 0
                            seo = ps2.tile([128, 1024], f32, tag="scores",
                                           name="seo", bufs=2)
                            for e in range(2):
                                nc.tensor.matmul(
                                    seo[:, 512 * e + off:512 * (e + 1)],
                                    kh[p][64 * e:64 * (e + 1),
                                          J * 128:(J + 1) * 128],
                                    qp[p][64 * e:64 * (e + 1),
                                          c * 512 + off:(c + 1) * 512],
                                    start=True, stop=True,
                                    tile_position=(64 * e, 0),
                                    skip_group_check=True,
                                )
                            # exp over both heads' halves in one op
                            et = sb_e.tile([128, 1024], dt, tag="exp",
                                           name="et")
                            ets[(p, c, J)] = et
                            src = seo[:].rearrange(
                                "p (e w) -> p e w", w=512)[:, :, off:] \
                                if off else seo[:]
                            dst = et[:].rearrange(
                                "p (e w) -> p e w", w=512)[:, :, off:] \
                                if off else et[:]
                            use_dve = (exp_idx % es_den) < es_num
                            exp_idx += 1
                            if use_dve:
                                nc.vector.tensor_scalar(
                                    dst.bitcast(i16), src, LAM, CEXP,
                                    mybir.AluOpType.mult, mybir.AluOpType.add)
                            else:
                                nc.scalar.activation(
                                    dst, src,
                                    mybir.ActivationFunctionType.Exp,
                                    bias=ebias[:], scale=EXP_SCALE,
                                )
                            if off or J == 4 * c:  # diagonal block: mask
                                trif = getattr(nc, tri_eng)
                                for e in range(2):
                                    sl = et[:, 512 * e + off:512 * e + off + 128]
                                    trif.tensor_mul(sl, sl, tri_s[:])
                            pending.append((p, c, J))
                            if len(pending) > 2:
                                av_emit(*pending.pop(0))
                for ent in pending:
                    av_emit(*ent)

            # ====== Phase 3: output projection (w_o stationary) ============
            with tc.tile_pool(name="ps5", bufs=2, space="PSUM") as ps5:
                for ms in range(D_MODEL // 128):
                    pu = [ps5.tile([128, 512], f32, tag=f"oproj{sp}",
                                   name=f"pu{sp}") for sp in range(NI)]
                    for p in range(2):
                        for sp in range(NI):
                            nc.tensor.matmul(
                                pu[sp][:],
                                wo_s[p][:, ms * 128:(ms + 1) * 128],
                                ot[p][:, sp * 512:(sp + 1) * 512],
                                start=(p == 0), stop=(p == 1),
                            )
                    us = sb_o.tile([128, N], dt, tag="ostage")
                    for sp in range(NI):
                        if sp % 2 == 0:
                            nc.vector.tensor_copy(
                                us[:, sp * 512:(sp + 1) * 512], pu[sp][:])
                        else:
                            nc.scalar.copy(
                                us[:, sp * 512:(sp + 1) * 512], pu[sp][:])
                    nc.sync.dma_start(outT_t[ms], us[:])

        if n_iters > 1:
            with tc.For_i(0, n_iters, 1):
                body()
        else:
            body()

    nc.compile()
    return nc


_CACHE = {}


def _get_program(dt_name: str, n_iters: int = 1):
    key = (dt_name, n_iters,
           os.environ.get("BASS_MHA_EXPSPLIT", "1/4"),
           os.environ.get("BASS_MHA_TRIENG", "vector"))
    if key not in _CACHE:
        _CACHE[key] = _build(dt_name, n_iters)
    return _CACHE[key]


def _np_dt(dt_name: str):
    if dt_name == "bf16":
        import ml_dtypes
        return ml_dtypes.bfloat16
    return np.float32


def make_in_maps(q, k, v, w_q, w_k, w_v, w_o, dt_name: str):
    """Build the 8 per-core input dicts (host-side shard + transpose)."""
    ndt = _np_dt(dt_name)
    tri = np.triu(np.ones((128, 128), np.float32)).astype(ndt)
    in_maps = []
    for b in range(B):
        xqT = np.ascontiguousarray(q[b].T).astype(ndt)
        xkT = np.ascontiguousarray(k[b].T).astype(ndt)
        xvT = np.ascontiguousarray(v[b].T).astype(ndt)
        for g in range(N_GROUPS):
            r0 = GD * g
            in_maps.append({
                "xqT": xqT,
                "xkT": xkT,
                "xvT": xvT,
                "wqT": np.ascontiguousarray(w_q[r0:r0 + GD, :].T).astype(ndt),
                "wkT": np.ascontiguousarray(w_k[r0:r0 + GD, :].T).astype(ndt),
                "wvT": np.ascontiguousarray(w_v[r0:r0 + GD, :].T).astype(ndt),
                "woT": np.ascontiguousarray(w_o[:, r0:r0 + GD].T).astype(ndt),
                "tri": tri,
            })
    return in_maps


def kernel(q, k, v, w_q, w_k, w_v, w_o):
    from concourse.bass_utils import run_bass_kernel_spmd

    dt_name = _DT
    nc = _get_program(dt_name)
    in_maps = make_in_maps(q, k, v, w_q, w_k, w_v, w_o, dt_name)
    res = run_bass_kernel_spmd(nc, in_maps, core_ids=list(range(N_CORES)))
    parts = [np.asarray(res.results[i]["outT"]).astype(np.float32)
             for i in range(N_CORES)]
    out = np.empty((B, N, D_MODEL), np.float32)
    for b in range(B):
        acc = parts[N_GROUPS * b]
        for g in range(1, N_GROUPS):
            acc = acc + parts[N_GROUPS * b + g]
        out[b] = acc.T
    return out
